# revision 25
# baseline (speedup 1.0000x reference)
"""Trainium2 Bass kernel for nn_CGAMotorModel.

Reference computes, for B=512, H=1024, D=5 multivector channels of Cl(4,1):
    W_x[b,h]  = sum_d x[b,d] o W_in[h,d]          (o = geometric product)
    h_free    = (1 - (1-dt)^n) * W_x              (closed form of the scan)
    out[b]    = sum_h h_free[b,h] o W_out[h]

By associativity/bilinearity of the geometric product this collapses to
    out[b] = c * sum_d x[b,d] o K_d,   K_d = sum_h W_in[h,d] o W_out[h]
with c = 1 - 0.9^10 (folded into x on the host, fp32-exact).

H-tensor-parallel over 8 cores (128 H-rows each); host sums the partial
outputs. Per core (all matmul inputs bf16, fp32 PSUM accumulate):
    S[r,(d,q)] = sum_h W_out[h,r] * W_in[h,(d,q)]         (1 matmul, K=128)
    K[g,d]     = sum_q C[q,:,:]^T @ S_q                   (32 matmuls, K=32)
    MT[m,5p+d] = sum_g C[p,g,m] * K[g,d]  per p           (32 matmuls, free=5,
                 psum free-dim offsets; partition base stays 0)
    M[(p,d),m] = PE-transpose of MT (2 transposes)
    out[b,m]   = xt[(p,d),b]^T @ M                        (8 matmuls)
where xt = X^T arrives via two DRAM->SBUF DmaTranspose ops (x host-permuted
to (p,d) column order and zero-padded to 192 cols so the tail window's rows
land at lhsT base partition 64 — matmul requires base 0/32/64), entirely off
the PE critical path. One bf16 Cayley table TBL[r, q*32+g] = C[q,r,g]
(+-1, exact in bf16) serves the K-step (lhsT slabs) and the MT-step
(lhsT slabs, read as TBL[g, 32p+m] = C[p,g,m]).

All matmul inputs are bf16 (1 PE cycle/row vs 4 for fp32); accumulation is
fp32 in PSUM, the free-phase constant c is folded into x on the host in
fp32, and the Cayley tables are exact in bf16, so the only error source is
bf16 rounding of x/W (measured rel err ~3.6e-3 vs the 2e-2 gate).
CoreSim cost-model time: 7527 ns vs the 13547 ns DRAM-bounce fp32 baseline.
Critical path: w DMA (~2.4us fixed latency) -> S -> ssb -> K -> ksb -> MT
-> mts -> PE transposes -> m12 -> final -> osb -> out DMA (~2.8us fixed
tail incl. 900ns DMA sem-prop + end-of-kernel drain/barrier).
"""

import numpy as np
import ml_dtypes

import concourse.bass as bass
import concourse.mybir as mybir
import concourse.tile as tile
from concourse import bacc
from concourse.bass_utils import run_bass_kernel_spmd
from concourse.masks import make_identity

B, H, D, MV = 512, 1024, 5, 32
N_CORES = 8
DT, N_FREE = 0.1, 10
C_SCALE = 1.0 - (1.0 - DT) ** N_FREE
F32 = mybir.dt.float32
BF16 = mybir.dt.bfloat16
BF = ml_dtypes.bfloat16


def _cayley_np() -> np.ndarray:
    """Cayley table for Cl(4,1), metric diag(1,1,1,1,-1). C[a,b,a^b] = sign."""
    metric = np.array([1.0, 1.0, 1.0, 1.0, -1.0], dtype=np.float32)
    C = np.zeros((32, 32, 32), dtype=np.float32)
    for a in range(32):
        for b in range(32):
            cnt = 0
            aa = a >> 1
            while aa:
                cnt += bin(aa & b).count("1")
                aa >>= 1
            s = -1.0 if (cnt & 1) else 1.0
            common = a & b
            for i in range(5):
                if (common >> i) & 1:
                    s *= metric[i]
            C[a, b, a ^ b] = s
    return C


# TBL[r, q*32+g] = C[q,r,g]; equally TBL[g, p*32+m] = C[p,g,m]. Entries are
# in {-1,0,1}: exact in bf16.
TBL = (
    np.ascontiguousarray(_cayley_np().transpose(1, 0, 2)).reshape(32, 1024).astype(BF)
)

# x column permutation (d,p) -> (p,d): x2[:, 5p+d] = x[:, 32d+p]
PERM = np.array([32 * d + p for p in range(32) for d in range(5)], dtype=np.int64)


def build_program() -> bass.Bass:
    # Bacc (not plain Bass): its compile pass moves multi-sem matmul waits
    # onto LdWeights — walrus rejects Matmult with >1 sync wait otherwise.
    nc = bacc.Bacc()
    x2 = nc.dram_tensor("x2", [B, 192], BF16, kind="ExternalInput")
    # wcat = [W_in.reshape(128,160) | W_out.reshape(128,32) | zero pad] for
    # this core's 128-row H-chunk; 256-col rows keep the DMA descriptor
    # elements at 512B, dodging the sub-512B 2x DMA latency penalty
    wcat = nc.dram_tensor("wcat", [128, 256], BF16, kind="ExternalInput")
    tbl = nc.dram_tensor("tbl", [32, 1024], BF16, kind="ExternalInput")
    # native layout [p, (t m)], b = 128t + p — host de-interleaves
    out = nc.dram_tensor("out", [128, 4 * MV], F32, kind="ExternalOutput")

    with tile.TileContext(nc) as tc:
        with (
            tc.tile_pool(name="sb", bufs=1) as sb,
            tc.tile_pool(name="ps", bufs=1, space="PSUM") as ps,
        ):
            # --- SP queue: w first (gates the whole S->K->M chain) ---
            w_sb = sb.tile([128, 256], BF16, tag="w_sb")
            nc.sync.dma_start(w_sb[:], wcat[:])
            # X^T via DMA-crossbar transposes straight from DRAM (16x128
            # tiles, bf16-only): xta rows = x2 cols 0:128; xtb rows 64:96
            # cover x2 cols 128:160 (window cols 64:192 keeps the free dim a
            # multiple of 128 and puts the live rows at base partition 64).
            xta = sb.tile([128, B], BF16, tag="xta")
            nc.sync.dma_start_transpose(xta[:], x2[:, 0:128])
            # --- ACT queue: table (gates K-step), then the second transpose ---
            tbl_sb = sb.tile([32, 1024], BF16, tag="tbl_sb")
            nc.scalar.dma_start(tbl_sb[:], tbl[:])
            xtb = sb.tile([128, B], BF16, tag="xtb")
            nc.scalar.dma_start_transpose(xtb[:], x2[:, 64:192])
            # identity for the PE transposes of MT (generated on Pool, no DMA)
            ident_sb = sb.tile([32, 32], BF16, tag="ident_sb")
            make_identity(nc, ident_sb[:])

            # --- S-step: one matmul, K=128 H-rows ---
            spsum = ps.tile([32, 160], F32, tag="spsum")
            nc.tensor.matmul(
                spsum[:], w_sb[:, 160:192], w_sb[:, 0:160], start=True, stop=True
            )
            ssb = sb.tile([32, 160], BF16, tag="ssb")
            nc.vector.tensor_copy(ssb[:], spsum[:])

            # --- K-step: K[g,d] = sum_q C[q]^T @ S_q ---
            kpsum = ps.tile([32, D], F32, tag="kpsum")
            for q in range(32):
                nc.tensor.matmul(
                    kpsum[:],
                    tbl_sb[:, 32 * q : 32 * (q + 1)],
                    ssb[:, q : 160 : 32],
                    start=(q == 0),
                    stop=(q == 31),
                )
            ksb = sb.tile([32, D], BF16, tag="ksb")
            nc.vector.tensor_copy(ksb[:], kpsum[:])

            # --- MT-step: MT[m, 5p+d] = sum_g C[p,g,m] * K[g,d] per p, at psum
            # FREE offset 5p (matmul psum partition base must be 0/32/64, so
            # the transposed layout with per-p free offsets is the legal one).
            mtp = ps.tile([32, 160], F32, tag="mtp")
            for p in range(32):
                nc.tensor.matmul(
                    mtp[:, 5 * p : 5 * p + 5],
                    tbl_sb[:, 32 * p : 32 * (p + 1)],
                    ksb[:],
                    start=True,
                    stop=True,
                )
            # mts padded to 192 cols (Pool zeroes the tail at t~0) so the
            # second transpose below can read a full 128-wide window ending
            # at col 192 — its output then covers every m12p row, and the
            # single m12 copy never reads uninitialized PSUM.
            mts = sb.tile([32, 192], BF16, tag="mts")
            nc.gpsimd.memset(mts[:, 160:192], 0.0)
            nc.vector.tensor_copy(mts[:, 0:160], mtp[:])

            # --- M = MT^T via two PE transposes ---
            # Both transposes land in ONE bf16 psum tile (the tail at
            # partition base 64 to match xtb's live rows — matmul requires
            # lhsT and rhs to share their base partition), so a single DVE
            # copy (2-byte 2x mode) moves M to SBUF.
            m12p = ps.tile([128, 2 * MV], BF16, tag="m12p")
            nc.tensor.transpose(m12p[:, 0:MV], mts[:, 0:128], ident_sb[:])
            # full-height window: rows 64:96 of this output are mts cols
            # 128:160, i.e. M rows 128:160; other rows are live-but-unused
            nc.tensor.transpose(m12p[:, MV : 2 * MV], mts[:, 64:192], ident_sb[:])
            m12 = sb.tile([128, 2 * MV], BF16, tag="m12")
            nc.vector.tensor_copy(m12[:], m12p[:])

            # --- final: out[b,m], 4 row-blocks of 128, all into ONE psum
            # bank (free-offset accumulation groups) -> ONE osb copy ---
            op = ps.tile([128, 4 * MV], F32, tag="opsum")
            for t in range(4):
                nc.tensor.matmul(
                    op[:, MV * t : MV * (t + 1)],
                    xta[:, 128 * t : 128 * (t + 1)],
                    m12[:, 0:MV],
                    start=True,
                    stop=False,
                )
                nc.tensor.matmul(
                    op[:, MV * t : MV * (t + 1)],
                    xtb[64:96, 128 * t : 128 * (t + 1)],
                    m12[64:96, MV : 2 * MV],
                    start=False,
                    stop=True,
                )
            osb = sb.tile([128, 4 * MV], F32, tag="osb")
            nc.vector.tensor_copy(osb[:], op[:])
            # row-split output: two half-size DMAs on separate queues
            nc.sync.dma_start(out[0:64, :], osb[0:64, :])
            nc.scalar.dma_start(out[64:128, :], osb[64:128, :])

    nc.finalize()
    return nc


_NC_CACHE: list = []


def make_inputs(x_mv: np.ndarray, W_in: np.ndarray, W_out: np.ndarray):
    """Host-side marshaling: fold c into x, permute columns to (p,d) order,
    cast matmul inputs to bf16, slice per-core H-chunks."""
    x = np.asarray(x_mv, dtype=np.float32)
    Wi = np.asarray(W_in, dtype=np.float32)
    Wo = np.asarray(W_out, dtype=np.float32)
    x2 = np.zeros((B, 192), dtype=BF)
    x2[:, 0 : D * MV] = (C_SCALE * x.reshape(B, D * MV))[:, PERM].astype(BF)
    wcat = np.zeros((H, 256), dtype=BF)
    wcat[:, 0 : D * MV] = Wi.reshape(H, D * MV).astype(BF)
    wcat[:, D * MV : D * MV + MV] = Wo.reshape(H, MV).astype(BF)
    return [
        {"x2": x2, "wcat": wcat[128 * c : 128 * (c + 1)], "tbl": TBL}
        for c in range(N_CORES)
    ]


def kernel(x_mv: np.ndarray, W_in: np.ndarray, W_out: np.ndarray) -> np.ndarray:
    if not _NC_CACHE:
        _NC_CACHE.append(build_program())
    nc = _NC_CACHE[0]

    in_maps = make_inputs(x_mv, W_in, W_out)
    try:
        res = run_bass_kernel_spmd(nc, in_maps, core_ids=list(range(N_CORES)))
    except Exception:
        # transient NRT/device hiccups have been observed; one retry
        res = run_bass_kernel_spmd(nc, in_maps, core_ids=list(range(N_CORES)))
    parts = [res.results[c]["out"] for c in range(N_CORES)]
    # device layout is [p, (t m)]; de-interleave to b = 128t + p
    out = np.sum(parts, axis=0, dtype=np.float32).reshape(128, 4, MV)
    out = out.transpose(1, 0, 2).reshape(B, MV)
    return np.ascontiguousarray(out, dtype=np.float32).reshape(B, 1, MV)
